# revision 23
# baseline (speedup 1.0000x reference)
"""Min-Euclidean-distance retrieval kernel for Trainium2 (8 NeuronCores).

Reference computation:
    x: [1, 2048, 512], y: [1, 65536, 512] (fp32)
    sq[p, r] = ||x_p||^2 + ||y_r||^2 - 2 <x_p, y_r>
    out = min over (p, r) of sqrt(max(sq, 0))

Sharding: the candidate pool (R) is split across 8 cores, 8192 candidates
each. The host pre-arranges both GEMM operands partition-major in fp8
(DoubleRow) with the -2 factor folded into x, so PSUM directly holds
H[r, p] = -2<x_p, y_r>.

The device reduces H to per-(lane, query) minima over the candidate
tiles. The norm terms never touch the device: queries are sorted by
||x||^2 and candidates by ||y||^2 (lane-major, so each output lane covers
64 y2-adjacent candidates), which makes host-side branch-and-bound
intervals tight. The host exactly recomputes the few surviving
(lane, query) cells in float64, so the result is exact as long as the
true argmin cell survives the +-SLACK pruning (~4.7 sigma of fp8 noise).

Engine plan (PE: 512 DoubleRow MMs, ~114us gap-free, is the roofline):
  - ScalarE drains 3 of 4 PSUM half-tiles to fp16 SBUF (1.2 GHz copies);
    DVE folds each copy into a per-(parity, tile-parity) fp16 accumulator
    with in-place min (2-byte SBUF operands run the DVE at 2x, ~690ns).
  - Every 4th half-tile skips ScalarE: one fused DVE tensor_tensor reads
    PSUM (only one PSUM input is legal) and the accumulator and writes
    the min in place at 1x.
  - The accumulators ship to DRAM raw; no on-device reduce at all.
This keeps ScalarE ~104us and DVE ~100us under the PE's ~115us, unlike
the v1 ACT-bias epilogue (ScalarE 127us serial) or a tensor_reduce-based
drain (DVE 146us: reduce never triggers the 2x mode, measured 1207ns).
Measured HW exec: ~132us at full clock (baseline 161us); the residual
over the MM window is framework preamble/DMA-spin-up/semaphore teardown.
"""

import sys

for _p in ("/opt/trn_rl_repo", "/root/.axon_site/_ro/trn_rl_repo"):
    if _p not in sys.path:
        sys.path.append(_p)

import ml_dtypes
import numpy as np

import concourse.bass as bass
import concourse.mybir as mybir
import concourse.tile as tile
from concourse import bacc, bass_utils

P = 2048          # queries
R = 65536         # candidates (full)
D = 512           # feature dim
NCORES = 8
R_LOC = R // NCORES      # 8192 candidates per core
P_CHUNKS = P // 512      # 4 chunks of queries (DMA + matmul granularity)
R_TILES = R_LOC // 128   # 64 stationary tiles of candidates
R_GROUPS = 16            # DMA granularity for y: 512 candidates per group
K_TILES = D // 128       # 4 contraction tiles
QGRP = 64                # query group size for the device-side min
NGRP_H = 1024 // QGRP    # 16 groups per query parity (half)
NGRP = P // QGRP         # 32 groups over all queries

# Bound slack for the host-side branch-and-bound: covers fp8 GEMM noise on
# H (sigma ~1 on a 512-dim dot) plus fp16 rounding of the staged copies.
SLACK = np.float64(8.0)

F32 = mybir.dt.float32
F16 = mybir.dt.float16
MM_DT = mybir.dt.float8e4
MM_NP = ml_dtypes.float8_e4m3

# Half-tile drain mode: every 4th half is drained by a fused DVE
# acc=min(PSUM, acc) op, the rest by ScalarE copies (load balance). The
# BIR verifier allows at most one PSUM input per DVE instruction. The
# second-to-last tile of each parity (last of chain 0) is also fused so
# the kernel tail never waits on a serial ScalarE copy + fold.
def _dve_half(h: int) -> bool:
    return h % 4 == 3 or h % R_TILES == R_TILES - 2


def _build_module() -> bass.Bass:
    nc = bacc.Bacc("TRN2", target_bir_lowering=False, debug=False)

    # Host-prepared layouts (partition-major, contiguous per partition):
    #   xt[q, c, k, j] = -2 * x_sorted[c*512 + j, k*128 + q]
    #   yt[q, g, k, s] = y_dev[g*512 + s, k*128 + q]
    # where y_dev[t*128 + l] = (per-core y2-sorted y)[l*64 + t].
    xt = nc.dram_tensor("xt", [128, P_CHUNKS, K_TILES, 512], MM_DT,
                        kind="ExternalInput")
    yt = nc.dram_tensor("yt", [128, R_GROUPS, K_TILES, 512], MM_DT,
                        kind="ExternalInput")
    # out[lane, parity, j, g, q] = min over candidate tiles t = j (mod 2)
    # of H for sorted query (parity*1024 + g*64 + q): the raw fp16
    # sub-accumulators. The group-reduce happens on the host, which also
    # gets exact per-query ||x||^2 bounds out of it.
    out = nc.dram_tensor("out", [128, 2, 2, NGRP_H, QGRP], F16,
                         kind="ExternalOutput")

    with tile.TileContext(nc) as tc:
        with (
            tc.tile_pool(name="big", bufs=1) as big,
            tc.tile_pool(name="node", bufs=6) as npool,
            tc.tile_pool(name="psum", bufs=4, space="PSUM") as psum,
        ):
            xt_sb = big.tile([128, P_CHUNKS, K_TILES, 512], MM_DT)
            yt_sb = big.tile([128, R_GROUPS, K_TILES, 512], MM_DT)
            # Two sub-accumulators per query parity: folds alternate
            # between them so the serial min-chain splits into two
            # independent chains (a single chain's per-op overheads stall
            # the PE ~358ns every 4 halves).
            acc = [
                [
                    big.tile([128, NGRP_H * QGRP], F16, name=f"acc{p}{j}")
                    for j in range(2)
                ]
                for p in range(2)
            ]

            # x on the scalar HWDGE ring, y on the sync ring (parallel).
            # The leading transfers are split at k-pair granularity so the
            # first matmul waits on only 128KB per ring; the trailing ones
            # are merged into big DMAs (each DMA costs a semaphore, and the
            # teardown sweep resets every semaphore at ~115ns apiece).
            nc.scalar.dma_start(xt_sb[:, 0, 0:2], xt.ap()[:, 0, 0:2])
            nc.sync.dma_start(yt_sb[:, 0, 0:2], yt.ap()[:, 0, 0:2])
            nc.scalar.dma_start(xt_sb[:, 0, 2:4], xt.ap()[:, 0, 2:4])
            nc.sync.dma_start(yt_sb[:, 0, 2:4], yt.ap()[:, 0, 2:4])
            nc.scalar.dma_start(xt_sb[:, 1], xt.ap()[:, 1])
            nc.sync.dma_start(yt_sb[:, 1], yt.ap()[:, 1])
            nc.sync.dma_start(yt_sb[:, 2], yt.ap()[:, 2])
            nc.sync.dma_start(yt_sb[:, 3], yt.ap()[:, 3])
            nc.sync.dma_start(yt_sb[:, 4:8], yt.ap()[:, 4:8])
            nc.sync.dma_start(yt_sb[:, 8:16], yt.ap()[:, 8:16])
            # The parity-1 query chunks are not consumed until the kernel
            # midpoint; queueing them LAST on the sync ring (FIFO per
            # queue) keeps their 512KB transfer out of the early HBM
            # contention window without costing ScalarE any instruction
            # time (the e4 variant that put this dma_start in the ScalarE
            # stream stalled the drain pipeline instead).
            nc.sync.dma_start(xt_sb[:, 2:4], xt.ap()[:, 2:4])

            acc_init = [[False, False], [False, False]]

            def mms(t: int, hh: int):
                """Fill one PSUM half-tile [128 cand x 1024 queries]."""
                g, o = t // 4, (t % 4) * 128
                pt = psum.tile([128, NGRP_H * QGRP], F32, name="pt")
                for ci in range(2):
                    c = hh * 2 + ci
                    for kk in range(K_TILES // 2):
                        nc.tensor.matmul(
                            pt[:, ci * 512 : (ci + 1) * 512],
                            lhsT=yt_sb[:, g, 2 * kk : 2 * kk + 2, o : o + 128],
                            rhs=xt_sb[:, c, 2 * kk : 2 * kk + 2, :],
                            start=(kk == 0),
                            stop=(kk == K_TILES // 2 - 1),
                            perf_mode=mybir.MatmulPerfMode.DoubleRow,
                        )
                return pt

            # Query-parity-outer order: all of parity 0's tiles finish at
            # the kernel midpoint, so its final reduce and output DMA
            # overlap parity 1's matmuls; only parity 1 drains in the tail.
            for hh in range(2):
                for t in range(R_TILES):
                    h = hh * R_TILES + t  # sequential half index
                    a = acc[hh][t % 2]
                    pt = mms(t, hh)
                    if not acc_init[hh][t % 2]:
                        # First producer of this chain seeds the
                        # accumulator via a ScalarE copy.
                        nc.scalar.activation(
                            out=a[:], in_=pt[:],
                            func=mybir.ActivationFunctionType.Copy)
                        acc_init[hh][t % 2] = True
                    elif _dve_half(h):
                        # Fused drain+fold: one 1x DVE pass reads PSUM and
                        # the fp16 accumulator and writes the min in place.
                        # The final tile drains in two halves so the tail
                        # only waits on the last chunk's matmuls.
                        if t == R_TILES - 1:
                            for ci in range(2):
                                s = slice(ci * 512, (ci + 1) * 512)
                                nc.vector.tensor_tensor(
                                    out=a[:, s], in0=pt[:, s],
                                    in1=a[:, s], op=mybir.AluOpType.min)
                        else:
                            nc.vector.tensor_tensor(
                                out=a[:], in0=pt[:], in1=a[:],
                                op=mybir.AluOpType.min)
                    else:
                        # ScalarE drains to fp16; DVE folds at its 2x
                        # (2-byte SBUF) rate.
                        node = npool.tile([128, NGRP_H * QGRP], F16, name="nd")
                        nc.scalar.activation(
                            out=node[:], in_=pt[:],
                            func=mybir.ActivationFunctionType.Copy)
                        nc.vector.tensor_tensor(
                            out=a[:], in0=a[:], in1=node[:],
                            op=mybir.AluOpType.min)
                    if t == R_TILES - 2:
                        # Chain 0 is complete: ship it while the last tile
                        # (chain 1) is still computing.
                        nc.sync.dma_start(out.ap()[:, hh, 0], acc[hh][0][:])
                nc.sync.dma_start(out.ap()[:, hh, 1], acc[hh][1][:])
    nc.compile()
    return nc


_module_cache: bass.Bass | None = None


def _get_module() -> bass.Bass:
    global _module_cache
    if _module_cache is None:
        _module_cache = _build_module()
    return _module_cache


def _to_partition_major(at: np.ndarray, nchunks: int) -> np.ndarray:
    """[D, W] transposed operand -> [128, nchunks, K_TILES, 512] fp8."""
    w = at.shape[1]
    a4 = at.reshape(K_TILES, 128, nchunks, w // nchunks)
    return np.ascontiguousarray(a4.transpose(1, 2, 0, 3).astype(MM_NP))


# Device slot rc = tile*128 + lane holds per-core-sorted candidate
# lane*64 + tile, so each output lane covers 64 y2-adjacent candidates.
_PERM = (np.arange(R_LOC) % 128) * (R_LOC // 128) + np.arange(R_LOC) // 128


def _prepare_inputs(x: np.ndarray, y: np.ndarray):
    """Host-side sharding/layout prep. Returns (per-core input maps,
    per-core y2-sorted candidate arrays). x must already be sorted by
    ||x||^2 (kernel() does the sort)."""
    xt = _to_partition_major((-2.0 * x).T, P_CHUNKS)
    in_maps, ysorts = [], []
    for c in range(NCORES):
        yc = y[c * R_LOC : (c + 1) * R_LOC]
        y2c = np.einsum("rd,rd->r", yc, yc, dtype=np.float64)
        ys = np.ascontiguousarray(yc[np.argsort(y2c, kind="stable")])
        ysorts.append(ys)
        yct = _to_partition_major(ys[_PERM].T, R_GROUPS)
        in_maps.append({"xt": xt, "yt": yct})
    return in_maps, ysorts


def _postprocess(xs: np.ndarray, ysorts: list, res: np.ndarray) -> np.ndarray:
    """Branch-and-bound on the device minima of H = -2<x,y>.

    xs: [P, D] queries sorted by ||x||^2; ysorts: per-core y2-sorted
    candidates; res: [NCORES, 128, 2, 2, NGRP_H, QGRP] fp16 sub-chain
    minima per query. Exact (float64) on the surviving cells."""
    xs64 = xs.astype(np.float64)
    x2 = np.einsum("pd,pd->p", xs64, xs64)

    ys64 = [ys.astype(np.float64) for ys in ysorts]
    y2s = np.stack([np.einsum("rd,rd->r", ys, ys) for ys in ys64])
    run = R_LOC // 128
    y2cell = y2s.reshape(NCORES, 128, run)
    y2cmin, y2cmax = y2cell.min(axis=2), y2cell.max(axis=2)

    # Min over the two sub-chains -> per-(core, lane, query) minima of H.
    hq = res.astype(np.float64).reshape(NCORES, 128, 2, 2, P // 2)
    hq = hq.min(axis=3).reshape(NCORES, 128, P)
    lb = hq + y2cmin[:, :, None] + x2[None, None, :] - SLACK
    ub = hq + y2cmax[:, :, None] + x2[None, None, :] + SLACK
    best_ub = ub.min()
    ks, ls, qs = np.nonzero(lb <= best_ub)

    best = np.inf
    for k, l, q in zip(ks, ls, qs):
        yc = ys64[k][l * run : (l + 1) * run]
        sq = x2[q] + y2cell[k, l] - 2.0 * (yc @ xs64[q])
        best = min(best, sq.min())
    return np.sqrt(np.float32(max(best, 0.0)))


def kernel(
    predicted_transaction_company: np.ndarray,
    future_transaction_companies_inc_current_data: np.ndarray,
) -> np.ndarray:
    x = np.asarray(predicted_transaction_company, dtype=np.float32)[0]
    y = np.asarray(future_transaction_companies_inc_current_data, dtype=np.float32)[0]

    # Sort queries by ||x||^2 so each group of 64 spans a narrow norm range
    # (tight branch-and-bound intervals). The min is order-invariant.
    order = np.argsort(np.einsum("pd,pd->p", x, x), kind="stable")
    xs = np.ascontiguousarray(x[order])

    nc = _get_module()
    in_maps, ysorts = _prepare_inputs(xs, y)
    res = bass_utils.run_bass_kernel_spmd(nc, in_maps, core_ids=list(range(NCORES)))
    accs = np.stack([r["out"] for r in res.results])
    return _postprocess(xs, ysorts, accs)


# revision 24
# speedup vs baseline: 1.0165x; 1.0165x over previous
"""Min-Euclidean-distance retrieval kernel for Trainium2 (8 NeuronCores).

Reference computation:
    x: [1, 2048, 512], y: [1, 65536, 512] (fp32)
    sq[p, r] = ||x_p||^2 + ||y_r||^2 - 2 <x_p, y_r>
    out = min over (p, r) of sqrt(max(sq, 0))

Sharding: the candidate pool (R) is split across 8 cores, 8192 candidates
each. The host pre-arranges both GEMM operands partition-major in fp8
(DoubleRow) with the -2 factor folded into x, so PSUM directly holds
H[r, p] = -2<x_p, y_r>.

The device reduces H to per-(lane, query) minima over the candidate
tiles. The norm terms never touch the device: queries are sorted by
||x||^2 and candidates by ||y||^2 (lane-major, so each output lane covers
64 y2-adjacent candidates), which makes host-side branch-and-bound
intervals tight. The host exactly recomputes the few surviving
(lane, query) cells in float64, so the result is exact as long as the
true argmin cell survives the +-SLACK pruning (~4.7 sigma of fp8 noise).

Engine plan (PE: 512 DoubleRow MMs, ~114us gap-free, is the roofline):
  - ScalarE drains 3 of 4 PSUM half-tiles to fp16 SBUF (1.2 GHz copies);
    DVE folds each copy into a per-(parity, tile-parity) fp16 accumulator
    with in-place min (2-byte SBUF operands run the DVE at 2x, ~690ns).
  - Every 4th half-tile skips ScalarE: one fused DVE tensor_tensor reads
    PSUM (only one PSUM input is legal) and the accumulator and writes
    the min in place at 1x.
  - The accumulators ship to DRAM raw; no on-device reduce at all.
This keeps ScalarE ~104us and DVE ~100us under the PE's ~115us, unlike
the v1 ACT-bias epilogue (ScalarE 127us serial) or a tensor_reduce-based
drain (DVE 146us: reduce never triggers the 2x mode, measured 1207ns).
Measured HW exec: ~132us at full clock (baseline 161us); the residual
over the MM window is framework preamble/DMA-spin-up/semaphore teardown.
"""

import sys

for _p in ("/opt/trn_rl_repo", "/root/.axon_site/_ro/trn_rl_repo"):
    if _p not in sys.path:
        sys.path.append(_p)

import ml_dtypes
import numpy as np

import concourse.bass as bass
import concourse.mybir as mybir
import concourse.tile as tile
from concourse import bacc, bass_utils

P = 2048          # queries
R = 65536         # candidates (full)
D = 512           # feature dim
NCORES = 8
R_LOC = R // NCORES      # 8192 candidates per core
P_CHUNKS = P // 512      # 4 chunks of queries (DMA + matmul granularity)
R_TILES = R_LOC // 128   # 64 stationary tiles of candidates
R_GROUPS = 16            # DMA granularity for y: 512 candidates per group
K_TILES = D // 128       # 4 contraction tiles
QGRP = 64                # query group size for the device-side min
NGRP_H = 1024 // QGRP    # 16 groups per query parity (half)
NGRP = P // QGRP         # 32 groups over all queries

# Bound slack for the host-side branch-and-bound: covers fp8 GEMM noise on
# H (sigma ~1 on a 512-dim dot) plus fp16 rounding of the staged copies.
SLACK = np.float64(8.0)

F32 = mybir.dt.float32
F16 = mybir.dt.float16
MM_DT = mybir.dt.float8e4
MM_NP = ml_dtypes.float8_e4m3

# Half-tile drain mode: every 4th half is drained by a fused DVE
# acc=min(PSUM, acc) op, the rest by ScalarE copies (load balance). The
# BIR verifier allows at most one PSUM input per DVE instruction. The
# second-to-last tile of each parity (last of chain 0) is also fused so
# the kernel tail never waits on a serial ScalarE copy + fold.
def _dve_half(h: int) -> bool:
    return h % 4 == 3 or h % R_TILES == R_TILES - 2


def _build_module() -> bass.Bass:
    nc = bacc.Bacc("TRN2", target_bir_lowering=False, debug=False)

    # Host-prepared layouts (partition-major, contiguous per partition):
    #   xt[q, c, k, j] = -2 * x_sorted[c*512 + j, k*128 + q]
    #   yt[q, g, k, s] = y_dev[g*512 + s, k*128 + q]
    # where y_dev[t*128 + l] = (per-core y2-sorted y)[l*64 + t].
    xt = nc.dram_tensor("xt", [128, P_CHUNKS, K_TILES, 512], MM_DT,
                        kind="ExternalInput")
    yt = nc.dram_tensor("yt", [128, R_GROUPS, K_TILES, 512], MM_DT,
                        kind="ExternalInput")
    # out[lane, parity, j, g, q] = min over candidate tiles t = j (mod 2)
    # of H for sorted query (parity*1024 + g*64 + q): the raw fp16
    # sub-accumulators. The group-reduce happens on the host, which also
    # gets exact per-query ||x||^2 bounds out of it.
    out = nc.dram_tensor("out", [128, 2, 2, NGRP_H, QGRP], F16,
                         kind="ExternalOutput")

    with tile.TileContext(nc) as tc:
        with (
            tc.tile_pool(name="big", bufs=1) as big,
            tc.tile_pool(name="node", bufs=6) as npool,
            tc.tile_pool(name="psum", bufs=4, space="PSUM") as psum,
        ):
            xt_sb = big.tile([128, P_CHUNKS, K_TILES, 512], MM_DT)
            yt_sb = big.tile([128, R_GROUPS, K_TILES, 512], MM_DT)
            # Two sub-accumulators per query parity: folds alternate
            # between them so the serial min-chain splits into two
            # independent chains (a single chain's per-op overheads stall
            # the PE ~358ns every 4 halves).
            acc = [
                [
                    big.tile([128, NGRP_H * QGRP], F16, name=f"acc{p}{j}")
                    for j in range(2)
                ]
                for p in range(2)
            ]

            # x on the scalar HWDGE ring, y on the sync ring (parallel).
            # The leading transfers are split at k-pair granularity so the
            # first matmul waits on only 128KB per ring; the trailing ones
            # are merged into big DMAs (each DMA costs a semaphore, and the
            # teardown sweep resets every semaphore at ~115ns apiece).
            nc.scalar.dma_start(xt_sb[:, 0, 0:2], xt.ap()[:, 0, 0:2])
            nc.sync.dma_start(yt_sb[:, 0, 0:2], yt.ap()[:, 0, 0:2])
            nc.scalar.dma_start(xt_sb[:, 0, 2:4], xt.ap()[:, 0, 2:4])
            nc.sync.dma_start(yt_sb[:, 0, 2:4], yt.ap()[:, 0, 2:4])
            nc.scalar.dma_start(xt_sb[:, 1], xt.ap()[:, 1])
            nc.sync.dma_start(yt_sb[:, 1], yt.ap()[:, 1])
            nc.scalar.dma_start(xt_sb[:, 2:4], xt.ap()[:, 2:4])
            nc.sync.dma_start(yt_sb[:, 2], yt.ap()[:, 2])
            nc.sync.dma_start(yt_sb[:, 3], yt.ap()[:, 3])
            nc.sync.dma_start(yt_sb[:, 4:8], yt.ap()[:, 4:8])
            nc.sync.dma_start(yt_sb[:, 8:16], yt.ap()[:, 8:16])

            acc_init = [[False, False], [False, False]]

            def mms(t: int, hh: int):
                """Fill one PSUM half-tile [128 cand x 1024 queries]."""
                g, o = t // 4, (t % 4) * 128
                pt = psum.tile([128, NGRP_H * QGRP], F32, name="pt")
                for ci in range(2):
                    c = hh * 2 + ci
                    for kk in range(K_TILES // 2):
                        nc.tensor.matmul(
                            pt[:, ci * 512 : (ci + 1) * 512],
                            lhsT=yt_sb[:, g, 2 * kk : 2 * kk + 2, o : o + 128],
                            rhs=xt_sb[:, c, 2 * kk : 2 * kk + 2, :],
                            start=(kk == 0),
                            stop=(kk == K_TILES // 2 - 1),
                            perf_mode=mybir.MatmulPerfMode.DoubleRow,
                        )
                return pt

            # Query-parity-outer order: all of parity 0's tiles finish at
            # the kernel midpoint, so its final reduce and output DMA
            # overlap parity 1's matmuls; only parity 1 drains in the tail.
            for hh in range(2):
                for t in range(R_TILES):
                    h = hh * R_TILES + t  # sequential half index
                    a = acc[hh][t % 2]
                    pt = mms(t, hh)
                    if not acc_init[hh][t % 2]:
                        # First producer of this chain seeds the
                        # accumulator via a ScalarE copy.
                        nc.scalar.activation(
                            out=a[:], in_=pt[:],
                            func=mybir.ActivationFunctionType.Copy)
                        acc_init[hh][t % 2] = True
                    elif _dve_half(h):
                        # Fused drain+fold: one 1x DVE pass reads PSUM and
                        # the fp16 accumulator and writes the min in place.
                        # The final tile drains in two halves so the tail
                        # only waits on the last chunk's matmuls.
                        if t == R_TILES - 1:
                            for ci in range(2):
                                s = slice(ci * 512, (ci + 1) * 512)
                                nc.vector.tensor_tensor(
                                    out=a[:, s], in0=pt[:, s],
                                    in1=a[:, s], op=mybir.AluOpType.min)
                        else:
                            nc.vector.tensor_tensor(
                                out=a[:], in0=pt[:], in1=a[:],
                                op=mybir.AluOpType.min)
                    else:
                        # ScalarE drains to fp16; DVE folds at its 2x
                        # (2-byte SBUF) rate.
                        node = npool.tile([128, NGRP_H * QGRP], F16, name="nd")
                        nc.scalar.activation(
                            out=node[:], in_=pt[:],
                            func=mybir.ActivationFunctionType.Copy)
                        nc.vector.tensor_tensor(
                            out=a[:], in0=a[:], in1=node[:],
                            op=mybir.AluOpType.min)
                    if t == R_TILES - 2:
                        # Chain 0 is complete: ship it while the last tile
                        # (chain 1) is still computing.
                        nc.sync.dma_start(out.ap()[:, hh, 0], acc[hh][0][:])
                nc.sync.dma_start(out.ap()[:, hh, 1], acc[hh][1][:])
    nc.compile()
    return nc


_module_cache: bass.Bass | None = None


def _get_module() -> bass.Bass:
    global _module_cache
    if _module_cache is None:
        _module_cache = _build_module()
    return _module_cache


def _to_partition_major(at: np.ndarray, nchunks: int) -> np.ndarray:
    """[D, W] transposed operand -> [128, nchunks, K_TILES, 512] fp8."""
    w = at.shape[1]
    a4 = at.reshape(K_TILES, 128, nchunks, w // nchunks)
    return np.ascontiguousarray(a4.transpose(1, 2, 0, 3).astype(MM_NP))


# Device slot rc = tile*128 + lane holds per-core-sorted candidate
# lane*64 + tile, so each output lane covers 64 y2-adjacent candidates.
_PERM = (np.arange(R_LOC) % 128) * (R_LOC // 128) + np.arange(R_LOC) // 128


def _prepare_inputs(x: np.ndarray, y: np.ndarray):
    """Host-side sharding/layout prep. Returns (per-core input maps,
    per-core y2-sorted candidate arrays). x must already be sorted by
    ||x||^2 (kernel() does the sort)."""
    xt = _to_partition_major((-2.0 * x).T, P_CHUNKS)
    in_maps, ysorts = [], []
    for c in range(NCORES):
        yc = y[c * R_LOC : (c + 1) * R_LOC]
        y2c = np.einsum("rd,rd->r", yc, yc, dtype=np.float64)
        ys = np.ascontiguousarray(yc[np.argsort(y2c, kind="stable")])
        ysorts.append(ys)
        yct = _to_partition_major(ys[_PERM].T, R_GROUPS)
        in_maps.append({"xt": xt, "yt": yct})
    return in_maps, ysorts


def _postprocess(xs: np.ndarray, ysorts: list, res: np.ndarray) -> np.ndarray:
    """Branch-and-bound on the device minima of H = -2<x,y>.

    xs: [P, D] queries sorted by ||x||^2; ysorts: per-core y2-sorted
    candidates; res: [NCORES, 128, 2, 2, NGRP_H, QGRP] fp16 sub-chain
    minima per query. Exact (float64) on the surviving cells."""
    xs64 = xs.astype(np.float64)
    x2 = np.einsum("pd,pd->p", xs64, xs64)

    ys64 = [ys.astype(np.float64) for ys in ysorts]
    y2s = np.stack([np.einsum("rd,rd->r", ys, ys) for ys in ys64])
    run = R_LOC // 128
    y2cell = y2s.reshape(NCORES, 128, run)
    y2cmin, y2cmax = y2cell.min(axis=2), y2cell.max(axis=2)

    # Min over the two sub-chains -> per-(core, lane, query) minima of H.
    hq = res.astype(np.float64).reshape(NCORES, 128, 2, 2, P // 2)
    hq = hq.min(axis=3).reshape(NCORES, 128, P)
    lb = hq + y2cmin[:, :, None] + x2[None, None, :] - SLACK
    ub = hq + y2cmax[:, :, None] + x2[None, None, :] + SLACK
    best_ub = ub.min()
    ks, ls, qs = np.nonzero(lb <= best_ub)

    best = np.inf
    for k, l, q in zip(ks, ls, qs):
        yc = ys64[k][l * run : (l + 1) * run]
        sq = x2[q] + y2cell[k, l] - 2.0 * (yc @ xs64[q])
        best = min(best, sq.min())
    return np.sqrt(np.float32(max(best, 0.0)))


def kernel(
    predicted_transaction_company: np.ndarray,
    future_transaction_companies_inc_current_data: np.ndarray,
) -> np.ndarray:
    x = np.asarray(predicted_transaction_company, dtype=np.float32)[0]
    y = np.asarray(future_transaction_companies_inc_current_data, dtype=np.float32)[0]

    # Sort queries by ||x||^2 so each group of 64 spans a narrow norm range
    # (tight branch-and-bound intervals). The min is order-invariant.
    order = np.argsort(np.einsum("pd,pd->p", x, x), kind="stable")
    xs = np.ascontiguousarray(x[order])

    nc = _get_module()
    in_maps, ysorts = _prepare_inputs(xs, y)
    res = bass_utils.run_bass_kernel_spmd(nc, in_maps, core_ids=list(range(NCORES)))
    accs = np.stack([r["out"] for r in res.results])
    return _postprocess(xs, ysorts, accs)


# revision 26
# speedup vs baseline: 1.0203x; 1.0038x over previous
"""Min-Euclidean-distance retrieval kernel for Trainium2 (8 NeuronCores).

Reference computation:
    x: [1, 2048, 512], y: [1, 65536, 512] (fp32)
    sq[p, r] = ||x_p||^2 + ||y_r||^2 - 2 <x_p, y_r>
    out = min over (p, r) of sqrt(max(sq, 0))

Sharding: the candidate pool (R) is split across 8 cores, 8192 candidates
each. The host pre-arranges both GEMM operands partition-major in fp8
(DoubleRow) with the -2 factor folded into x, so PSUM directly holds
H[r, p] = -2<x_p, y_r>.

The device reduces H to per-(lane, query) minima over the candidate
tiles. The norm terms never touch the device: queries are sorted by
||x||^2 and candidates by ||y||^2 (lane-major, so each output lane covers
64 y2-adjacent candidates), which makes host-side branch-and-bound
intervals tight. The host exactly recomputes the few surviving
(lane, query) cells in float64, so the result is exact as long as the
true argmin cell survives the +-SLACK pruning (~4.7 sigma of fp8 noise).

Engine plan (PE: 512 DoubleRow MMs, ~114us gap-free, is the roofline):
  - ScalarE drains 3 of 4 PSUM half-tiles to fp16 SBUF (1.2 GHz copies);
    DVE folds each copy into a per-(parity, tile-parity) fp16 accumulator
    with in-place min (2-byte SBUF operands run the DVE at 2x, ~690ns).
  - Every 4th half-tile skips ScalarE: one fused DVE tensor_tensor reads
    PSUM (only one PSUM input is legal) and the accumulator and writes
    the min in place at 1x.
  - The accumulators ship to DRAM raw; no on-device reduce at all.
This keeps ScalarE ~104us and DVE ~100us under the PE's ~115us, unlike
the v1 ACT-bias epilogue (ScalarE 127us serial) or a tensor_reduce-based
drain (DVE 146us: reduce never triggers the 2x mode, measured 1207ns).
Measured HW exec: ~132us at full clock (baseline 161us); the residual
over the MM window is framework preamble/DMA-spin-up/semaphore teardown.
"""

import sys

for _p in ("/opt/trn_rl_repo", "/root/.axon_site/_ro/trn_rl_repo"):
    if _p not in sys.path:
        sys.path.append(_p)

import ml_dtypes
import numpy as np

import concourse.bass as bass
import concourse.mybir as mybir
import concourse.tile as tile
from concourse import bacc, bass_utils

P = 2048          # queries
R = 65536         # candidates (full)
D = 512           # feature dim
NCORES = 8
R_LOC = R // NCORES      # 8192 candidates per core
P_CHUNKS = P // 512      # 4 chunks of queries (DMA + matmul granularity)
R_TILES = R_LOC // 128   # 64 stationary tiles of candidates
R_GROUPS = 16            # DMA granularity for y: 512 candidates per group
K_TILES = D // 128       # 4 contraction tiles
QGRP = 64                # query group size for the device-side min
NGRP_H = 1024 // QGRP    # 16 groups per query parity (half)
NGRP = P // QGRP         # 32 groups over all queries

# Bound slack for the host-side branch-and-bound: covers fp8 GEMM noise on
# H (sigma ~1 on a 512-dim dot) plus fp16 rounding of the staged copies.
SLACK = np.float64(8.0)

F32 = mybir.dt.float32
F16 = mybir.dt.float16
MM_DT = mybir.dt.float8e4
MM_NP = ml_dtypes.float8_e4m3

# Half-tile drain mode: every 4th half is drained by a fused DVE
# acc=min(PSUM, acc) op, the rest by ScalarE copies (load balance). The
# BIR verifier allows at most one PSUM input per DVE instruction. The
# second-to-last tile of each parity (last of chain 0) is also fused so
# the kernel tail never waits on a serial ScalarE copy + fold.
def _dve_half(h: int) -> bool:
    return h % 4 == 3 or h % R_TILES == R_TILES - 2


def _build_module() -> bass.Bass:
    nc = bacc.Bacc("TRN2", target_bir_lowering=False, debug=False)

    # Host-prepared layouts (partition-major, contiguous per partition):
    #   xt[q, c, k, j] = -2 * x_sorted[c*512 + j, k*128 + q]
    #   yt[q, g, k, s] = y_dev[g*512 + s, k*128 + q]
    # where y_dev[t*128 + l] = (per-core y2-sorted y)[l*64 + t].
    xt = nc.dram_tensor("xt", [128, P_CHUNKS, K_TILES, 512], MM_DT,
                        kind="ExternalInput")
    yt = nc.dram_tensor("yt", [128, R_GROUPS, K_TILES, 512], MM_DT,
                        kind="ExternalInput")
    # out[lane, parity, j, g, q] = min over candidate tiles t = j (mod 2)
    # of H for sorted query (parity*1024 + g*64 + q): the raw fp16
    # sub-accumulators. The group-reduce happens on the host, which also
    # gets exact per-query ||x||^2 bounds out of it.
    out = nc.dram_tensor("out", [128, 2, 2, NGRP_H, QGRP], F16,
                         kind="ExternalOutput")

    with tile.TileContext(nc) as tc:
        with (
            tc.tile_pool(name="big", bufs=1) as big,
            tc.tile_pool(name="node", bufs=6) as npool,
            tc.tile_pool(name="psum", bufs=4, space="PSUM") as psum,
        ):
            xt_sb = big.tile([128, P_CHUNKS, K_TILES, 512], MM_DT)
            yt_sb = big.tile([128, R_GROUPS, K_TILES, 512], MM_DT)
            # Two sub-accumulators per query parity: folds alternate
            # between them so the serial min-chain splits into two
            # independent chains (a single chain's per-op overheads stall
            # the PE ~358ns every 4 halves).
            acc = [
                [
                    big.tile([128, NGRP_H * QGRP], F16, name=f"acc{p}{j}")
                    for j in range(2)
                ]
                for p in range(2)
            ]

            # x on the scalar HWDGE ring, y on the sync ring (parallel).
            # The leading transfers are split at k-pair granularity so the
            # first matmul waits on only 128KB per ring; the trailing ones
            # are merged into big DMAs (each DMA costs a semaphore, and the
            # teardown sweep resets every semaphore at ~115ns apiece).
            nc.scalar.dma_start(xt_sb[:, 0, 0:2], xt.ap()[:, 0, 0:2])
            nc.sync.dma_start(yt_sb[:, 0, 0:2], yt.ap()[:, 0, 0:2])
            nc.scalar.dma_start(xt_sb[:, 0, 2:4], xt.ap()[:, 0, 2:4])
            nc.sync.dma_start(yt_sb[:, 0, 2:4], yt.ap()[:, 0, 2:4])
            nc.scalar.dma_start(xt_sb[:, 1], xt.ap()[:, 1])
            nc.sync.dma_start(yt_sb[:, 1], yt.ap()[:, 1])
            nc.scalar.dma_start(xt_sb[:, 2:4], xt.ap()[:, 2:4])
            nc.sync.dma_start(yt_sb[:, 2], yt.ap()[:, 2])
            nc.sync.dma_start(yt_sb[:, 3], yt.ap()[:, 3])
            nc.sync.dma_start(yt_sb[:, 4:8], yt.ap()[:, 4:8])
            nc.sync.dma_start(yt_sb[:, 8:16], yt.ap()[:, 8:16])

            acc_init = [[False, False], [False, False]]

            def mms(t: int, hh: int):
                """Fill one PSUM half-tile [128 cand x 1024 queries]."""
                g, o = t // 4, (t % 4) * 128
                pt = psum.tile([128, NGRP_H * QGRP], F32, name="pt")
                for ci in range(2):
                    c = hh * 2 + ci
                    for kk in range(K_TILES // 2):
                        nc.tensor.matmul(
                            pt[:, ci * 512 : (ci + 1) * 512],
                            lhsT=yt_sb[:, g, 2 * kk : 2 * kk + 2, o : o + 128],
                            rhs=xt_sb[:, c, 2 * kk : 2 * kk + 2, :],
                            start=(kk == 0),
                            stop=(kk == K_TILES // 2 - 1),
                            perf_mode=mybir.MatmulPerfMode.DoubleRow,
                        )
                return pt

            # Query-parity-outer order: all of parity 0's tiles finish at
            # the kernel midpoint, so its final reduce and output DMA
            # overlap parity 1's matmuls; only parity 1 drains in the tail.
            for hh in range(2):
                for t in range(R_TILES):
                    h = hh * R_TILES + t  # sequential half index
                    a = acc[hh][t % 2]
                    pt = mms(t, hh)
                    if not acc_init[hh][t % 2]:
                        # First producer of this chain seeds the
                        # accumulator via a ScalarE copy.
                        nc.scalar.activation(
                            out=a[:], in_=pt[:],
                            func=mybir.ActivationFunctionType.Copy)
                        acc_init[hh][t % 2] = True
                    elif _dve_half(h):
                        # Fused drain+fold: one 1x DVE pass reads PSUM and
                        # the fp16 accumulator and writes the min in place.
                        # The final tile drains in two halves so the tail
                        # only waits on the last chunk's matmuls.
                        if t == R_TILES - 1:
                            for ci in range(2):
                                s = slice(ci * 512, (ci + 1) * 512)
                                nc.vector.tensor_tensor(
                                    out=a[:, s], in0=pt[:, s],
                                    in1=a[:, s], op=mybir.AluOpType.min)
                        else:
                            nc.vector.tensor_tensor(
                                out=a[:], in0=pt[:], in1=a[:],
                                op=mybir.AluOpType.min)
                    else:
                        # ScalarE drains to fp16; DVE folds at its 2x
                        # (2-byte SBUF) rate.
                        node = npool.tile([128, NGRP_H * QGRP], F16, name="nd")
                        nc.scalar.activation(
                            out=node[:], in_=pt[:],
                            func=mybir.ActivationFunctionType.Copy)
                        nc.vector.tensor_tensor(
                            out=a[:], in0=a[:], in1=node[:],
                            op=mybir.AluOpType.min)
                    if t == R_TILES - 2:
                        # Chain 0 is complete: ship it while the last tile
                        # (chain 1) is still computing.
                        nc.sync.dma_start(out.ap()[:, hh, 0], acc[hh][0][:])
                nc.sync.dma_start(out.ap()[:, hh, 1], acc[hh][1][:])
    nc.compile()
    return nc


_module_cache: bass.Bass | None = None


def _get_module() -> bass.Bass:
    global _module_cache
    if _module_cache is None:
        _module_cache = _build_module()
    return _module_cache


def _to_partition_major(at: np.ndarray, nchunks: int) -> np.ndarray:
    """[D, W] transposed operand -> [128, nchunks, K_TILES, 512] fp8."""
    w = at.shape[1]
    a4 = at.reshape(K_TILES, 128, nchunks, w // nchunks)
    return np.ascontiguousarray(a4.transpose(1, 2, 0, 3).astype(MM_NP))


# Device slot rc = tile*128 + lane holds per-core-sorted candidate
# lane*64 + tile, so each output lane covers 64 y2-adjacent candidates.
_PERM = (np.arange(R_LOC) % 128) * (R_LOC // 128) + np.arange(R_LOC) // 128


def _prepare_inputs(x: np.ndarray, y: np.ndarray):
    """Host-side sharding/layout prep. Returns (per-core input maps,
    per-core y2-sorted candidate arrays). x must already be sorted by
    ||x||^2 (kernel() does the sort)."""
    xt = _to_partition_major((-2.0 * x).T, P_CHUNKS)
    in_maps, ysorts = [], []
    for c in range(NCORES):
        yc = y[c * R_LOC : (c + 1) * R_LOC]
        y2c = np.einsum("rd,rd->r", yc, yc, dtype=np.float64)
        ys = np.ascontiguousarray(yc[np.argsort(y2c, kind="stable")])
        ysorts.append(ys)
        yct = _to_partition_major(ys[_PERM].T, R_GROUPS)
        in_maps.append({"xt": xt, "yt": yct})
    return in_maps, ysorts


def _postprocess(xs: np.ndarray, ysorts: list, res: np.ndarray) -> np.ndarray:
    """Branch-and-bound on the device minima of H = -2<x,y>.

    xs: [P, D] queries sorted by ||x||^2; ysorts: per-core y2-sorted
    candidates; res: [NCORES, 128, 2, 2, NGRP_H, QGRP] fp16 sub-chain
    minima per query. Exact (float64) on the surviving cells."""
    xs64 = xs.astype(np.float64)
    x2 = np.einsum("pd,pd->p", xs64, xs64)

    ys64 = [ys.astype(np.float64) for ys in ysorts]
    y2s = np.stack([np.einsum("rd,rd->r", ys, ys) for ys in ys64])
    run = R_LOC // 128
    y2cell = y2s.reshape(NCORES, 128, run)
    y2cmin, y2cmax = y2cell.min(axis=2), y2cell.max(axis=2)

    # Min over the two sub-chains -> per-(core, lane, query) minima of H.
    hq = res.astype(np.float64).reshape(NCORES, 128, 2, 2, P // 2)
    hq = hq.min(axis=3).reshape(NCORES, 128, P)
    lb = hq + y2cmin[:, :, None] + x2[None, None, :] - SLACK
    ub = hq + y2cmax[:, :, None] + x2[None, None, :] + SLACK
    best_ub = ub.min()
    ks, ls, qs = np.nonzero(lb <= best_ub)

    best = np.inf
    for k, l, q in zip(ks, ls, qs):
        yc = ys64[k][l * run : (l + 1) * run]
        sq = x2[q] + y2cell[k, l] - 2.0 * (yc @ xs64[q])
        best = min(best, sq.min())
    return np.sqrt(np.float32(max(best, 0.0)))


def kernel(
    predicted_transaction_company: np.ndarray,
    future_transaction_companies_inc_current_data: np.ndarray,
) -> np.ndarray:
    x = np.asarray(predicted_transaction_company, dtype=np.float32)[0]
    y = np.asarray(future_transaction_companies_inc_current_data, dtype=np.float32)[0]

    # Sort queries by ||x||^2 so each group of 64 spans a narrow norm range
    # (tight branch-and-bound intervals). The min is order-invariant.
    order = np.argsort(np.einsum("pd,pd->p", x, x), kind="stable")
    xs = np.ascontiguousarray(x[order])

    nc = _get_module()
    in_maps, ysorts = _prepare_inputs(xs, y)
    res = bass_utils.run_bass_kernel_spmd(nc, in_maps, core_ids=list(range(NCORES)))
    accs = np.stack([r["out"] for r in res.results])
    return _postprocess(xs, ysorts, accs)
